# revision 22
# baseline (speedup 1.0000x reference)
"""ContraNorm kernel for 8x Trainium2 NeuronCores (Bass/Tile).

Computes, for x [8192, 512] fp32 (gamma/beta [512]):
    xn  = x / max(||x||_row, eps)
    sim = xn @ xn.T
    sim = softmax(sim, axis=1) + softmax(sim, axis=0)
    y   = x - 0.1 * (sim @ x)
    out = LayerNorm(y) * gamma + beta          (eps = 1e-6)

Key math used by the kernel:
  * sim entries are cosine similarities in [-1, 1], so exp() never
    overflows and softmax needs no max-subtraction:
        row_softmax[i,j] = E[i,j] / r_i,  E = exp(sim),  r_i = sum_j E[i,j]
  * E is symmetric, so column sums equal row sums:  c_j = r_j.
        sim' = E * (1/r_i + 1/r_j)   (elementwise)
  * Row-shard across 8 cores. Core q owns rows [q*1024, (q+1)*1024).
    It computes E^T tiles T[j, i] (j = all 8192 on partitions,
    i = its 1024 rows on free dim); ACT exp accumulates partial row
    sums; one 32KB AllReduce + ReduceScatter of those partials gives
    every core the full r (for 1/r_j, per-partition) and its own slice
    (for 1/r_i, partition-broadcast) without any core-id branching.
"""

import sys

if "/opt/trn_rl_repo" not in sys.path:
    sys.path.insert(0, "/opt/trn_rl_repo")

import ml_dtypes
import numpy as np

import concourse.bass as bass
import concourse.tile as tile
from concourse import bacc, mybir
from concourse.bass_utils import run_bass_kernel_spmd

N = 8192
D = 512
N_CORES = 8
B = N // N_CORES          # 1024 rows per core
P = 128
JC = N // P               # 64 j-chunks
IT = B // 512             # 2 i-halves of the per-core block
ISUB = B // P             # 8 output row-subtiles
KO = D // P               # 4 contraction chunks
SCALE = 0.1
LN_EPS = 1e-6

F32 = mybir.dt.float32
BF16 = mybir.dt.bfloat16
AF = mybir.ActivationFunctionType


def build_kernel(reps=1):
    nc = bacc.Bacc("TRN2", target_bir_lowering=False, debug=False,
                   num_devices=N_CORES)

    # ---- I/O ----
    xT = nc.dram_tensor("xT", [D, N], BF16, kind="ExternalInput")       # x.T
    xb = nc.dram_tensor("xb", [N, D], BF16, kind="ExternalInput")       # x bf16
    xTq = nc.dram_tensor("xTq", [D, B], BF16, kind="ExternalInput")     # x.T own cols
    xq = nc.dram_tensor("xq", [B, D], F32, kind="ExternalInput")        # own rows fp32
    gamma = nc.dram_tensor("gamma", [D], F32, kind="ExternalInput")
    beta = nc.dram_tensor("beta", [D], F32, kind="ExternalInput")
    out = nc.dram_tensor("out", [B, D], F32, kind="ExternalOutput")

    xT_v = xT.ap().rearrange("(ko p) j -> p ko j", p=P)       # [128, 4, 8192]
    xTq_v = xTq.ap().rearrange("(ko p) i -> p ko i", p=P)     # [128, 4, 1024]
    xb_v = xb.ap().rearrange("(c p) d -> p c d", p=P)         # [128, 64, 512]
    xq_v = xq.ap().rearrange("(c p) d -> p c d", p=P)         # [128, 8, 512]

    with tile.TileContext(nc) as tc:
        for rep in range(reps):
            _body(nc, tc, xT_v, xTq_v, xb_v, xq_v, gamma, beta, out, sfx=f"r{rep}")
    nc.compile()
    return nc


def _body(nc, tc, xT_v, xTq_v, xb_v, xq_v, gamma, beta, out, sfx="", ablate=()):
    from contextlib import ExitStack
    ablate = set(ablate)
    ctx = ExitStack()
    with ctx:
        persist = ctx.enter_context(tc.tile_pool(name=f"persist{sfx}", bufs=1))
        small = ctx.enter_context(tc.tile_pool(name=f"small{sfx}", bufs=2))
        stream = ctx.enter_context(tc.tile_pool(name=f"stream{sfx}", bufs=3))
        etile = ctx.enter_context(tc.tile_pool(name=f"etile{sfx}", bufs=3))
        pwork = ctx.enter_context(tc.tile_pool(name=f"pwork{sfx}", bufs=2))
        lnw = ctx.enter_context(tc.tile_pool(name=f"lnw{sfx}", bufs=2))
        dram = ctx.enter_context(tc.tile_pool(name=f"dram{sfx}", bufs=1, space="DRAM"))

        # ---------- load persistent operands ----------
        xTq_sb = persist.tile([P, KO, B], BF16)       # 1 MB
        nc.gpsimd.dma_start(xTq_sb[:], xTq_v[:])

        gamma_b = persist.tile([P, D], F32)
        nc.gpsimd.dma_start(gamma_b[:], bass.AP(tensor=gamma, offset=0,
                                                ap=[[0, P], [1, D]]))
        beta_b = persist.tile([P, D], F32)
        nc.gpsimd.dma_start(beta_b[:], bass.AP(tensor=beta, offset=0,
                                               ap=[[0, P], [1, D]]))

        # ---------- inverse norms ----------
        # invn of own rows first (gates phase A rhs), from fp32
        xq_sb = persist.tile([P, ISUB, D], F32)       # own rows fp32 (2 MB)
        nc.gpsimd.dma_start(xq_sb[:], xq_v[:])
        sq_scr = small.tile([P, D], BF16, tag="sqscr")
        ss_q = persist.tile([P, ISUB], F32)
        for t in range(ISUB):
            nc.scalar.activation(out=sq_scr[:], in_=xq_sb[:, t, :], func=AF.Square,
                                 accum_out=ss_q[:, t:t + 1])
        n_q = small.tile([P, ISUB], F32, tag="nq")
        nc.scalar.activation(out=n_q[:], in_=ss_q[:], func=AF.Sqrt)
        invn_q = small.tile([P, ISUB], F32, tag="invnq")
        nc.vector.reciprocal(out=invn_q[:], in_=n_q[:])
        d_invnq = dram.tile([B], F32)
        nc.gpsimd.dma_start(d_invnq.rearrange("(c p) -> p c", p=P), invn_q[:])
        invnq_b = persist.tile([P, B], F32)
        nc.gpsimd.dma_start(invnq_b[:], bass.AP(tensor=d_invnq.tensor,
                                                offset=d_invnq.offset,
                                                ap=[[0, P], [1, B]]))

        # normalized own columns: xnTq[d, i] = xTq[d, i] * invn_q[i]
        xnTq_sb = persist.tile([P, KO, B], BF16)
        for k in range(KO):
            nc.vector.tensor_tensor(out=xnTq_sb[:, k, :], in0=xTq_sb[:, k, :],
                                    in1=invnq_b[:], op=mybir.AluOpType.mult)

        # invn_all[p, c] = 1/||x_row(c*128+p)|| for all rows: every core has
        # its own 1024 fp32 inverse norms in d_invnq — AllGather them
        rg = [list(range(N_CORES))]
        c_nall = dram.tile([N], F32)
        nc.gpsimd.collective_compute("AllGather", mybir.AluOpType.bypass,
                                     replica_groups=rg,
                                     ins=[d_invnq.opt()], outs=[c_nall.opt()])
        invn_all = persist.tile([P, JC], F32)
        nc.gpsimd.dma_start(invn_all[:], c_nall.rearrange("(c p) -> p c", p=P))

        # ---------- phase A: E^T tiles + partial row sums ----------
        e_dram = dram.tile([JC, P, B], BF16)          # 16.8 MB scratch
        sacc = persist.tile([P, JC], F32)             # accum_out slots
        if "phase_a" in ablate:
            nc.vector.memset(sacc[:], 1.0)
        with tc.tile_pool(name=f"psum_a{sfx}", bufs=4, space="PSUM") as psum_a:
            for jq in range((JC // 4) if "phase_a" not in ablate else 0):
                xt4 = stream.tile([P, KO, 512], BF16, tag="xt4")
                nc.sync.dma_start(xt4[:], xT_v[:, :, jq * 512:(jq + 1) * 512])
                etq = etile.tile([P, 4, B], BF16, tag="etq")
                for jj in range(4):
                    jc = jq * 4 + jj
                    pt = psum_a.tile([P, B], F32, tag="ph_a")
                    for k in range(KO):
                        for it in range(IT):
                            nc.tensor.matmul(
                                pt[:, it * 512:(it + 1) * 512],
                                xt4[:, k, jj * P:(jj + 1) * P],
                                xnTq_sb[:, k, it * 512:(it + 1) * 512],
                                start=(k == 0), stop=(k == KO - 1))
                    nc.scalar.activation(out=etq[:, jj, :], in_=pt[:],
                                         func=AF.Exp,
                                         scale=invn_all[:, jc:jc + 1],
                                         accum_out=sacc[:, jc:jc + 1])
                nc.sync.dma_start(e_dram[jq * 4:(jq + 1) * 4], etq[:])

        # ---------- collectives: r = global row sums ----------
        c_in = dram.tile([N], F32)
        nc.gpsimd.dma_start(c_in.rearrange("(c p) -> p c", p=P), sacc[:])
        c_ar = dram.tile([N], F32)
        c_rs = dram.tile([B], F32)
        nc.gpsimd.collective_compute("AllReduce", mybir.AluOpType.add,
                                     replica_groups=rg,
                                     ins=[c_in.opt()], outs=[c_ar.opt()])
        nc.gpsimd.collective_compute("ReduceScatter", mybir.AluOpType.add,
                                     replica_groups=rg,
                                     ins=[c_in.opt()], outs=[c_rs.opt()])
        r_all = small.tile([P, JC], F32, tag="rall")
        nc.gpsimd.dma_start(r_all[:], c_ar.rearrange("(c p) -> p c", p=P))
        invr_f = small.tile([P, JC], F32, tag="invrf")
        nc.vector.reciprocal(out=invr_f[:], in_=r_all[:])
        invr_all = persist.tile([P, JC], BF16)
        nc.scalar.copy(out=invr_all[:], in_=invr_f[:])
        rq_b = small.tile([P, B], F32, tag="rqb")
        nc.gpsimd.dma_start(rq_b[:], bass.AP(tensor=c_rs.tensor, offset=c_rs.offset,
                                             ap=[[0, P], [1, B]]))
        invrq_f = small.tile([P, B], F32, tag="invrqf")
        nc.vector.reciprocal(out=invrq_f[:], in_=rq_b[:])
        invrq_b = persist.tile([P, B], BF16)
        nc.scalar.copy(out=invrq_b[:], in_=invrq_f[:])

        # ---------- phase C: x_neg = P^T.T @ x ----------
        psum_c = ctx.enter_context(
            tc.tile_pool(name=f"psum_c{sfx}", bufs=1, space="PSUM"))
        acc = [psum_c.tile([P, D], F32, tag=f"acc{i}", name=f"acc{i}")
               for i in range(ISUB)]
        n_jq = (JC // 4) if "phase_c" not in ablate else 1
        for jq in range(n_jq):
            etq = etile.tile([P, 4, B], BF16, tag="etq")
            nc.sync.dma_start(etq[:], e_dram[jq * 4:(jq + 1) * 4])
            xb4 = stream.tile([P, 4, D], BF16, tag="xb4_c")
            nc.gpsimd.dma_start(xb4[:], xb_v[:, jq * 4:(jq + 1) * 4, :])
            if "mp" in ablate:
                p_t = etq
            else:
                m_t = pwork.tile([P, 4, B], BF16, tag="mt")
                nc.vector.tensor_tensor(
                    out=m_t[:],
                    in0=invrq_b[:, None, :].to_broadcast((P, 4, B)),
                    in1=invr_all[:, jq * 4:(jq + 1) * 4, None].to_broadcast(
                        (P, 4, B)),
                    op=mybir.AluOpType.add)
                p_t = pwork.tile([P, 4, B], BF16, tag="pt")
                nc.vector.tensor_tensor(out=p_t[:], in0=etq[:], in1=m_t[:],
                                        op=mybir.AluOpType.mult)
            for jj in range(4):
                last = (jq == n_jq - 1) and (jj == 3)
                for i in range(ISUB):
                    nc.tensor.matmul(acc[i][:],
                                     p_t[:, jj, i * P:(i + 1) * P],
                                     xb4[:, jj, :],
                                     start=(jq == 0 and jj == 0), stop=last)

        # ---------- tail: y = xq - 0.1*x_neg ; LayerNorm ----------
        eps_t = small.tile([P, 1], F32, tag="eps")
        nc.vector.memset(eps_t[:], LN_EPS)
        y_all = persist.tile([P, ISUB, D], F32)
        mv_all = persist.tile([P, ISUB, 2], F32)
        for i in range(ISUB):
            nc.vector.tensor_scalar(out=y_all[:, i, :], in0=acc[i][:],
                                    scalar1=-SCALE,
                                    scalar2=None, op0=mybir.AluOpType.mult)
            nc.vector.tensor_tensor(out=y_all[:, i, :], in0=y_all[:, i, :],
                                    in1=xq_sb[:, i, :], op=mybir.AluOpType.add)
            stats = lnw.tile([P, 6], F32, tag="stats")
            nc.vector.bn_stats(out=stats[:], in_=y_all[:, i, :])
            nc.vector.bn_aggr(out=mv_all[:, i, :], in_=stats[:])
        std_all = small.tile([P, ISUB], F32, tag="stdall")
        nc.scalar.activation(out=std_all[:], in_=mv_all[:, :, 1], func=AF.Sqrt,
                             bias=eps_t[:])
        rstd_all = small.tile([P, ISUB], F32, tag="rstdall")
        nc.vector.reciprocal(out=rstd_all[:], in_=std_all[:])
        o_t = persist.tile([P, ISUB, D], F32)
        for i in range(ISUB):
            nc.vector.tensor_scalar(out=o_t[:, i, :], in0=y_all[:, i, :],
                                    scalar1=mv_all[:, i, 0:1],
                                    scalar2=rstd_all[:, i:i + 1],
                                    op0=mybir.AluOpType.subtract,
                                    op1=mybir.AluOpType.mult)
            nc.vector.tensor_tensor(out=o_t[:, i, :], in0=o_t[:, i, :],
                                    in1=gamma_b[:], op=mybir.AluOpType.mult)
            nc.vector.tensor_tensor(out=o_t[:, i, :], in0=o_t[:, i, :],
                                    in1=beta_b[:], op=mybir.AluOpType.add)
        nc.sync.dma_start(out.ap().rearrange("(c p) d -> p c d", p=P), o_t[:])


_CACHE = {}


def _get_nc():
    if "nc" not in _CACHE:
        _CACHE["nc"] = build_kernel()
    return _CACHE["nc"]


def make_in_maps(x, gamma, beta):
    x = np.asarray(x, dtype=np.float32)
    xT_bf = np.ascontiguousarray(x.T).astype(ml_dtypes.bfloat16)
    xb_bf = x.astype(ml_dtypes.bfloat16)
    gamma = np.asarray(gamma, dtype=np.float32)
    beta = np.asarray(beta, dtype=np.float32)
    in_maps = []
    for q in range(N_CORES):
        sl = slice(q * B, (q + 1) * B)
        in_maps.append({
            "xT": xT_bf,
            "xb": xb_bf,
            "xTq": np.ascontiguousarray(xT_bf[:, sl]),
            "xq": np.ascontiguousarray(x[sl]),
            "gamma": gamma,
            "beta": beta,
        })
    return in_maps


def kernel(x, gamma, beta):
    nc = _get_nc()
    in_maps = make_in_maps(x, gamma, beta)
    res = run_bass_kernel_spmd(nc, in_maps, core_ids=list(range(N_CORES)))
    out = np.concatenate([res.results[q]["out"] for q in range(N_CORES)], axis=0)
    return out.astype(np.float32)


if __name__ == "__main__":
    rng = np.random.default_rng(0)
    x = rng.standard_normal((N, D), dtype=np.float32)
    gamma = np.ones(D, np.float32)
    beta = np.zeros(D, np.float32)
    o = kernel(x, gamma, beta)
    print("out", o.shape, o.dtype, float(np.abs(o).mean()))


# revision 23
# speedup vs baseline: 1.1496x; 1.1496x over previous
"""ContraNorm kernel for 8x Trainium2 NeuronCores (Bass/Tile).

Computes, for x [8192, 512] fp32 (gamma/beta [512]):
    xn  = x / max(||x||_row, eps)
    sim = xn @ xn.T
    sim = softmax(sim, axis=1) + softmax(sim, axis=0)
    y   = x - 0.1 * (sim @ x)
    out = LayerNorm(y) * gamma + beta          (eps = 1e-6)

Key math used by the kernel:
  * sim entries are cosine similarities in [-1, 1], so exp() never
    overflows and softmax needs no max-subtraction:
        row_softmax[i,j] = E[i,j] / r_i,  E = exp(sim),  r_i = sum_j E[i,j]
  * E is symmetric, so column sums equal row sums:  c_j = r_j.
        sim' = E * (1/r_i + 1/r_j)   (elementwise)
  * Row-shard across 8 cores. Core q owns rows [q*1024, (q+1)*1024).
    It computes E^T tiles T[j, i] (j = all 8192 on partitions,
    i = its 1024 rows on free dim); ACT exp accumulates partial row
    sums; one 32KB AllReduce + ReduceScatter of those partials gives
    every core the full r (for 1/r_j, per-partition) and its own slice
    (for 1/r_i, partition-broadcast) without any core-id branching.
"""

import sys

if "/opt/trn_rl_repo" not in sys.path:
    sys.path.insert(0, "/opt/trn_rl_repo")

import ml_dtypes
import numpy as np

import concourse.bass as bass
import concourse.tile as tile
from concourse import bacc, mybir
from concourse.bass_utils import run_bass_kernel_spmd

N = 8192
D = 512
N_CORES = 8
B = N // N_CORES          # 1024 rows per core
P = 128
JC = N // P               # 64 j-chunks
IT = B // 512             # 2 i-halves of the per-core block
ISUB = B // P             # 8 output row-subtiles
KO = D // P               # 4 contraction chunks
SCALE = 0.1
LN_EPS = 1e-6

F32 = mybir.dt.float32
BF16 = mybir.dt.bfloat16
AF = mybir.ActivationFunctionType


def build_kernel(reps=1):
    nc = bacc.Bacc("TRN2", target_bir_lowering=False, debug=False,
                   num_devices=N_CORES)

    # ---- I/O ----
    xT = nc.dram_tensor("xT", [D, N], BF16, kind="ExternalInput")       # x.T
    xb = nc.dram_tensor("xb", [N, D], BF16, kind="ExternalInput")       # x bf16
    xTq = nc.dram_tensor("xTq", [D, B], BF16, kind="ExternalInput")     # x.T own cols
    xq = nc.dram_tensor("xq", [B, D], F32, kind="ExternalInput")        # own rows fp32
    gamma = nc.dram_tensor("gamma", [D], F32, kind="ExternalInput")
    beta = nc.dram_tensor("beta", [D], F32, kind="ExternalInput")
    out = nc.dram_tensor("out", [B, D], F32, kind="ExternalOutput")

    xT_v = xT.ap().rearrange("(ko p) j -> p ko j", p=P)       # [128, 4, 8192]
    xTq_v = xTq.ap().rearrange("(ko p) i -> p ko i", p=P)     # [128, 4, 1024]
    xb_v = xb.ap().rearrange("(c p) d -> p c d", p=P)         # [128, 64, 512]
    xq_v = xq.ap().rearrange("(c p) d -> p c d", p=P)         # [128, 8, 512]

    with tile.TileContext(nc) as tc:
        for rep in range(reps):
            _body(nc, tc, xT_v, xTq_v, xb_v, xq_v, gamma, beta, out, sfx=f"r{rep}")
    nc.compile()
    return nc


def _body(nc, tc, xT_v, xTq_v, xb_v, xq_v, gamma, beta, out, sfx="", ablate=()):
    from contextlib import ExitStack
    ablate = set(ablate)
    ctx = ExitStack()
    with ctx:
        persist = ctx.enter_context(tc.tile_pool(name=f"persist{sfx}", bufs=1))
        small = ctx.enter_context(tc.tile_pool(name=f"small{sfx}", bufs=2))
        stream = ctx.enter_context(tc.tile_pool(name=f"stream{sfx}", bufs=3))
        etile = ctx.enter_context(tc.tile_pool(name=f"etile{sfx}", bufs=3))
        pwork = ctx.enter_context(tc.tile_pool(name=f"pwork{sfx}", bufs=2))
        lnw = ctx.enter_context(tc.tile_pool(name=f"lnw{sfx}", bufs=2))
        dram = ctx.enter_context(tc.tile_pool(name=f"dram{sfx}", bufs=1, space="DRAM"))

        # ---------- load persistent operands ----------
        xTq_sb = persist.tile([P, KO, B], BF16)       # 1 MB
        nc.gpsimd.dma_start(xTq_sb[:], xTq_v[:])

        gamma_b = persist.tile([P, D], F32)
        nc.gpsimd.dma_start(gamma_b[:], bass.AP(tensor=gamma, offset=0,
                                                ap=[[0, P], [1, D]]))
        beta_b = persist.tile([P, D], F32)
        nc.gpsimd.dma_start(beta_b[:], bass.AP(tensor=beta, offset=0,
                                               ap=[[0, P], [1, D]]))

        # ---------- inverse norms ----------
        # invn of own rows first (gates phase A rhs), from fp32
        xq_sb = persist.tile([P, ISUB, D], F32)       # own rows fp32 (2 MB)
        nc.gpsimd.dma_start(xq_sb[:], xq_v[:])
        sq_scr = small.tile([P, D], BF16, tag="sqscr")
        ss_q = persist.tile([P, ISUB], F32)
        for t in range(ISUB):
            nc.scalar.activation(out=sq_scr[:], in_=xq_sb[:, t, :], func=AF.Square,
                                 accum_out=ss_q[:, t:t + 1])
        n_q = small.tile([P, ISUB], F32, tag="nq")
        nc.scalar.activation(out=n_q[:], in_=ss_q[:], func=AF.Sqrt)
        invn_q = small.tile([P, ISUB], F32, tag="invnq")
        nc.vector.reciprocal(out=invn_q[:], in_=n_q[:])
        d_invnq = dram.tile([B], F32)
        nc.gpsimd.dma_start(d_invnq.rearrange("(c p) -> p c", p=P), invn_q[:])
        invnq_b = persist.tile([P, B], F32)
        nc.gpsimd.dma_start(invnq_b[:], bass.AP(tensor=d_invnq.tensor,
                                                offset=d_invnq.offset,
                                                ap=[[0, P], [1, B]]))

        # normalized own columns: xnTq[d, i] = xTq[d, i] * invn_q[i]
        xnTq_sb = persist.tile([P, KO, B], BF16)
        for k in range(KO):
            nc.vector.tensor_tensor(out=xnTq_sb[:, k, :], in0=xTq_sb[:, k, :],
                                    in1=invnq_b[:], op=mybir.AluOpType.mult)

        # invn_all[p, c] = 1/||x_row(c*128+p)|| from bf16 x, in groups of 8
        # chunks so early j-chunks' exp is not gated on the whole pass
        rg = [list(range(N_CORES))]
        ss_all = persist.tile([P, JC], F32)
        n_all = persist.tile([P, JC], F32)
        invn_all = persist.tile([P, JC], F32)
        for g in range(JC // 8):
            xb8 = stream.tile([P, 8, D], BF16, tag="xb8")
            nc.sync.dma_start(xb8[:], xb_v[:, g * 8:(g + 1) * 8, :])
            for c in range(8):
                nc.scalar.activation(out=sq_scr[:], in_=xb8[:, c, :],
                                     func=AF.Square,
                                     accum_out=ss_all[:, g * 8 + c:g * 8 + c + 1])
            nc.scalar.activation(out=n_all[:, g * 8:(g + 1) * 8],
                                 in_=ss_all[:, g * 8:(g + 1) * 8], func=AF.Sqrt)
            nc.vector.reciprocal(out=invn_all[:, g * 8:(g + 1) * 8],
                                 in_=n_all[:, g * 8:(g + 1) * 8])

        # ---------- phase A: E^T tiles + partial row sums ----------
        e_dram = dram.tile([JC, P, B], BF16)          # 16.8 MB scratch
        sacc = persist.tile([P, JC], F32)             # accum_out slots
        if "phase_a" in ablate:
            nc.vector.memset(sacc[:], 1.0)
        with tc.tile_pool(name=f"psum_a{sfx}", bufs=4, space="PSUM") as psum_a:
            for jq in range((JC // 4) if "phase_a" not in ablate else 0):
                xt4 = stream.tile([P, KO, 512], BF16, tag="xt4")
                nc.sync.dma_start(xt4[:], xT_v[:, :, jq * 512:(jq + 1) * 512])
                etq = etile.tile([P, 4, B], BF16, tag="etq")
                for jj in range(4):
                    jc = jq * 4 + jj
                    pt = psum_a.tile([P, B], F32, tag="ph_a")
                    for k in range(KO):
                        for it in range(IT):
                            nc.tensor.matmul(
                                pt[:, it * 512:(it + 1) * 512],
                                xt4[:, k, jj * P:(jj + 1) * P],
                                xnTq_sb[:, k, it * 512:(it + 1) * 512],
                                start=(k == 0), stop=(k == KO - 1))
                    nc.scalar.activation(out=etq[:, jj, :], in_=pt[:],
                                         func=AF.Exp,
                                         scale=invn_all[:, jc:jc + 1],
                                         accum_out=sacc[:, jc:jc + 1])
                nc.sync.dma_start(e_dram[jq * 4:(jq + 1) * 4], etq[:])

        # ---------- collectives: r = global row sums ----------
        c_in = dram.tile([N], F32)
        nc.gpsimd.dma_start(c_in.rearrange("(c p) -> p c", p=P), sacc[:])
        c_ar = dram.tile([N], F32)
        c_rs = dram.tile([B], F32)
        nc.gpsimd.collective_compute("AllReduce", mybir.AluOpType.add,
                                     replica_groups=rg,
                                     ins=[c_in.opt()], outs=[c_ar.opt()])
        nc.gpsimd.collective_compute("ReduceScatter", mybir.AluOpType.add,
                                     replica_groups=rg,
                                     ins=[c_in.opt()], outs=[c_rs.opt()])
        r_all = small.tile([P, JC], F32, tag="rall")
        nc.gpsimd.dma_start(r_all[:], c_ar.rearrange("(c p) -> p c", p=P))
        invr_f = small.tile([P, JC], F32, tag="invrf")
        nc.vector.reciprocal(out=invr_f[:], in_=r_all[:])
        invr_all = persist.tile([P, JC], BF16)
        nc.scalar.copy(out=invr_all[:], in_=invr_f[:])
        rq_b = small.tile([P, B], F32, tag="rqb")
        nc.gpsimd.dma_start(rq_b[:], bass.AP(tensor=c_rs.tensor, offset=c_rs.offset,
                                             ap=[[0, P], [1, B]]))
        invrq_f = small.tile([P, B], F32, tag="invrqf")
        nc.vector.reciprocal(out=invrq_f[:], in_=rq_b[:])
        invrq_b = persist.tile([P, B], BF16)
        nc.scalar.copy(out=invrq_b[:], in_=invrq_f[:])

        # ---------- phase C: x_neg = P^T.T @ x ----------
        psum_c = ctx.enter_context(
            tc.tile_pool(name=f"psum_c{sfx}", bufs=1, space="PSUM"))
        acc = [psum_c.tile([P, D], F32, tag=f"acc{i}", name=f"acc{i}")
               for i in range(ISUB)]
        n_jq = (JC // 4) if "phase_c" not in ablate else 1
        for jq in range(n_jq):
            etq = etile.tile([P, 4, B], BF16, tag="etq")
            nc.sync.dma_start(etq[:], e_dram[jq * 4:(jq + 1) * 4])
            xb4 = stream.tile([P, 4, D], BF16, tag="xb4_c")
            nc.gpsimd.dma_start(xb4[:], xb_v[:, jq * 4:(jq + 1) * 4, :])
            if "mp" in ablate:
                p_t = etq
            else:
                m_t = pwork.tile([P, 4, B], BF16, tag="mt")
                nc.vector.tensor_tensor(
                    out=m_t[:],
                    in0=invrq_b[:, None, :].to_broadcast((P, 4, B)),
                    in1=invr_all[:, jq * 4:(jq + 1) * 4, None].to_broadcast(
                        (P, 4, B)),
                    op=mybir.AluOpType.add)
                p_t = pwork.tile([P, 4, B], BF16, tag="pt")
                nc.vector.tensor_tensor(out=p_t[:], in0=etq[:], in1=m_t[:],
                                        op=mybir.AluOpType.mult)
            for jj in range(4):
                last = (jq == n_jq - 1) and (jj == 3)
                for i in range(ISUB):
                    nc.tensor.matmul(acc[i][:],
                                     p_t[:, jj, i * P:(i + 1) * P],
                                     xb4[:, jj, :],
                                     start=(jq == 0 and jj == 0), stop=last)

        # ---------- tail: y = xq - 0.1*x_neg ; LayerNorm ----------
        eps_t = small.tile([P, 1], F32, tag="eps")
        nc.vector.memset(eps_t[:], LN_EPS)
        y_all = persist.tile([P, ISUB, D], F32)
        mv_all = persist.tile([P, ISUB, 2], F32)
        for i in range(ISUB):
            nc.vector.tensor_scalar(out=y_all[:, i, :], in0=acc[i][:],
                                    scalar1=-SCALE,
                                    scalar2=None, op0=mybir.AluOpType.mult)
            nc.vector.tensor_tensor(out=y_all[:, i, :], in0=y_all[:, i, :],
                                    in1=xq_sb[:, i, :], op=mybir.AluOpType.add)
            stats = lnw.tile([P, 6], F32, tag="stats")
            nc.vector.bn_stats(out=stats[:], in_=y_all[:, i, :])
            nc.vector.bn_aggr(out=mv_all[:, i, :], in_=stats[:])
        std_all = small.tile([P, ISUB], F32, tag="stdall")
        nc.scalar.activation(out=std_all[:], in_=mv_all[:, :, 1], func=AF.Sqrt,
                             bias=eps_t[:])
        rstd_all = small.tile([P, ISUB], F32, tag="rstdall")
        nc.vector.reciprocal(out=rstd_all[:], in_=std_all[:])
        o_t = persist.tile([P, ISUB, D], F32)
        for i in range(ISUB):
            nc.vector.tensor_scalar(out=o_t[:, i, :], in0=y_all[:, i, :],
                                    scalar1=mv_all[:, i, 0:1],
                                    scalar2=rstd_all[:, i:i + 1],
                                    op0=mybir.AluOpType.subtract,
                                    op1=mybir.AluOpType.mult)
            nc.vector.tensor_tensor(out=o_t[:, i, :], in0=o_t[:, i, :],
                                    in1=gamma_b[:], op=mybir.AluOpType.mult)
            nc.vector.tensor_tensor(out=o_t[:, i, :], in0=o_t[:, i, :],
                                    in1=beta_b[:], op=mybir.AluOpType.add)
        nc.sync.dma_start(out.ap().rearrange("(c p) d -> p c d", p=P), o_t[:])


_CACHE = {}


def _get_nc():
    if "nc" not in _CACHE:
        _CACHE["nc"] = build_kernel()
    return _CACHE["nc"]


def make_in_maps(x, gamma, beta):
    x = np.asarray(x, dtype=np.float32)
    xT_bf = np.ascontiguousarray(x.T).astype(ml_dtypes.bfloat16)
    xb_bf = x.astype(ml_dtypes.bfloat16)
    gamma = np.asarray(gamma, dtype=np.float32)
    beta = np.asarray(beta, dtype=np.float32)
    in_maps = []
    for q in range(N_CORES):
        sl = slice(q * B, (q + 1) * B)
        in_maps.append({
            "xT": xT_bf,
            "xb": xb_bf,
            "xTq": np.ascontiguousarray(xT_bf[:, sl]),
            "xq": np.ascontiguousarray(x[sl]),
            "gamma": gamma,
            "beta": beta,
        })
    return in_maps


def kernel(x, gamma, beta):
    nc = _get_nc()
    in_maps = make_in_maps(x, gamma, beta)
    res = run_bass_kernel_spmd(nc, in_maps, core_ids=list(range(N_CORES)))
    out = np.concatenate([res.results[q]["out"] for q in range(N_CORES)], axis=0)
    return out.astype(np.float32)


if __name__ == "__main__":
    rng = np.random.default_rng(0)
    x = rng.standard_normal((N, D), dtype=np.float32)
    gamma = np.ones(D, np.float32)
    beta = np.zeros(D, np.float32)
    o = kernel(x, gamma, beta)
    print("out", o.shape, o.dtype, float(np.abs(o).mean()))
